# revision 42
# baseline (speedup 1.0000x reference)
"""Trainium2 Bass kernel for nn_CausalGraphLearner (gnn_message_passing).

Math (B=16, V=128, H=128):
  adj[j,i]  = sigmoid(L[j,i]) * (1-eye) * sigmoid(5*(t[j]-t[i]))
  C[b,j,h]  = variables[b,j] @ W1[:H]          (cause part)
  E[b,i,h]  = variables[b,i] @ W1[H:]          (effect part)
  hidden    = relu(E[b,i,h] + C[b,j,h] + b1[h])
  weighted[b,i,h] = sum_j adj[j,i] * hidden[b,i,j,h]
  effects   = weighted @ W2 + b2 * colsum(adj)[i]
  returns (effects, adj)

Sharding: 8 cores, core c owns effect-variable slice i in [16c, 16c+16)
for all batches (cause-side data replicated).  All core-dependent
indexing is resolved host-side by passing pre-sliced / pre-transposed /
pre-broadcast tensors so a single SPMD program serves all cores and the
device spends no time on layout shuffling.

Per-core schedule: each generation tile (batch b, 8 effect vars = 2 PSUM
banks) gets its pre-relu hidden built by f32r matmuls (cause part
streamed N=512 against a 4x-replicated W1a, effect part broadcast across
partitions via a K=1 ones-row matmul against per-partition-flattened E
rows), relu-evicted to SBUF bf16 on ScalarE/VectorE alternately, then
contracted over j (lhsT = hidden block, moving operand = adjacency
column) into weighted^T PSUM column accumulators; a final pair of
matmuls per 128-pair chunk applies W2 and the rank-1 b2*colsum(adj) bias.
"""

import sys

sys.path.insert(0, "/opt/trn_rl_repo")

import numpy as np

B, V, H = 16, 128, 128
N_CORES = 8
OWN = V // N_CORES  # 16 effect vars per core

# ---- pack1 column layout (SBUF [128, NPACK], host-prepacked) ----
_off = 0


def _seg(n):
    global _off
    s = _off
    _off += n
    return s


OFF_W2 = _seg(H)  # 128
OFF_ADJL = _seg(V)  # 128
OFF_TROW = _seg(V)  # 128: temporal order broadcast rows
OFF_B2 = _seg(H)  # 128: b2 in partition row 0 only
NPACK = _off

# ---- packs: small fp32 pack with adjacency-critical columns (first DMA) ----
_off = 0
OFF_ADJLO = _seg(OWN)  # 16
OFF_TCOL = _seg(1)  # 1: temporal order as column
OFF_TOROW = _seg(OWN)  # 16: own temporal order broadcast rows
OFF_DMASK = _seg(OWN)  # 16: 1 - onehot(own diag)
NPACKS = _off

# ---- packb column layout (SBUF bf16 [128, NPACKB], host-prepacked) ----
_off = 0
OFF_VTO = _seg(B * OWN)  # 256: vars_own^T  [hh, (b,il)]
OFF_W1B = _seg(H)  # 128: W1[H:] (hh-major)
OFF_W1A4 = _seg(4 * H)  # 512: W1[:H] replicated 4x along cols
OFF_B1R = _seg(H)  # 128: b1 broadcast to all partitions (bf16 ok: |b1|~0.01)
NPACKB = _off

_CACHE = {}


def _build_program():
    import concourse.bass as bass
    import concourse.mybir as mybir
    from concourse import bacc
    from concourse.tile import TileContext

    f32 = mybir.dt.float32
    f32r = mybir.dt.float32r
    bf16 = mybir.dt.bfloat16
    AF = mybir.ActivationFunctionType
    ALU = mybir.AluOpType

    nc = bacc.Bacc("TRN2", debug=False, num_devices=N_CORES)

    # ---------------- DRAM I/O ----------------
    d_pack = nc.dram_tensor("pack1", [128, NPACK], f32, kind="ExternalInput")
    d_packs = nc.dram_tensor("packs", [128, NPACKS], f32, kind="ExternalInput")
    d_packb = nc.dram_tensor("packb", [128, NPACKB], bf16, kind="ExternalInput")
    d_vt = nc.dram_tensor("vt", [128, B * V], bf16, kind="ExternalInput")
    d_eff = nc.dram_tensor("eff", [B * OWN, H], f32, kind="ExternalOutput")
    d_adj_out = nc.dram_tensor("adj_out", [V, V], f32, kind="ExternalOutput")

    with TileContext(nc) as tc:
        with (
            tc.tile_pool(name="singles", bufs=1) as singles,
            tc.tile_pool(name="work", bufs=3) as work,
            tc.tile_pool(name="hid", bufs=8) as hidp,
            tc.tile_pool(name="psgen", bufs=3, space="PSUM") as psgen,
            tc.tile_pool(name="pswacc", bufs=1, space="PSUM") as pswacc,
        ):
            # ---------------- constants ----------------
            ones_col = singles.tile([128, 1], f32)
            nc.gpsimd.memset(ones_col, 1.0)
            ones_sb = singles.tile([128, 128], bf16)
            nc.gpsimd.memset(ones_sb, 1.0)

            # PE warmup: dependency-free matmuls at t=0 ramp the PE clock
            # (HAM / p-state) to full speed while the input DMAs are in
            # flight, so the first real matmuls run warm.
            wu = psgen.tile([128, 128], f32, tag="gen", name="wu")
            for _ in range(14):
                nc.tensor.matmul(wu, lhsT=ones_sb, rhs=ones_sb, start=True, stop=True)

            # ---------------- input DMAs ----------------
            packs = singles.tile([128, NPACKS], f32)
            nc.scalar.dma_start(out=packs, in_=d_packs.ap())
            packb = singles.tile([128, NPACKB], bf16)
            nc.sync.dma_start(out=packb, in_=d_packb.ap())
            VT = singles.tile([128, B * V], bf16)  # [hh, (b,j)] (host-transposed)
            for quarter in range(4):
                eng = nc.scalar if quarter % 2 == 0 else nc.sync
                eng.dma_start(
                    out=VT[:, quarter * 512 : (quarter + 1) * 512],
                    in_=d_vt.ap()[:, quarter * 512 : (quarter + 1) * 512],
                )
            pack = singles.tile([128, NPACK], f32)
            nc.scalar.dma_start(out=pack, in_=d_pack.ap())

            VTo = packb[:, OFF_VTO : OFF_VTO + B * OWN]
            W1b = packb[:, OFF_W1B : OFF_W1B + H]
            W1a4 = packb[:, OFF_W1A4 : OFF_W1A4 + 4 * H]
            b1row = packb[:, OFF_B1R : OFF_B1R + H]
            W2sb = pack[:, OFF_W2 : OFF_W2 + H]
            adjl_sb = pack[:, OFF_ADJL : OFF_ADJL + V]
            adjlo_sb = packs[:, OFF_ADJLO : OFF_ADJLO + OWN]
            t_col = packs[:, OFF_TCOL : OFF_TCOL + 1]
            t_row = pack[:, OFF_TROW : OFF_TROW + V]
            to_row = packs[:, OFF_TOROW : OFF_TOROW + OWN]
            dmask_sb = packs[:, OFF_DMASK : OFF_DMASK + OWN]
            b2row = pack[0:1, OFF_B2 : OFF_B2 + H]

            eflat = singles.tile([128, 4 * OWN * H], bf16)  # row 32k holds E rows
            # for b in [4k,4k+4); all other rows zero so a K=32 all-ones
            # matmul picks out just the E row (K=1 bf16 matmuls hang the PE).
            # Zeroed first thing: no deps, runs while the input DMAs fly.
            half_f = OWN * H  # fp32-view half width (view is [128, 4096])
            nc.vector.memset(eflat.bitcast(f32)[:, 0:half_f], 0.0)
            nc.gpsimd.memset(eflat.bitcast(f32)[:, half_f : 2 * half_f], 0.0)

            # ---------------- E (own rows, + b1) ----------------
            # Echunk[m][(b,il)-128, h] = vars_own @ W1b + b1, then flatten so
            # the rows for 4 consecutive batches live in one partition
            # (0/32/64/96), usable as the K=1 moving operand of the broadcast
            # matmul.
            echunk = singles.tile([128, 2 * H], bf16)
            for m in range(2):
                ps_e = psgen.tile([128, H], f32, tag="gen", name="ps_e")
                nc.tensor.matmul(
                    ps_e,
                    lhsT=VTo[:, m * 128 : (m + 1) * 128],
                    rhs=W1b,
                    start=True,
                    stop=True,
                )
                nc.vector.tensor_tensor(
                    out=echunk[:, m * H : (m + 1) * H],
                    in0=ps_e,
                    in1=b1row,
                    op=ALU.add,
                )
            for k in range(4):
                src = echunk[
                    (k % 2) * 64 : (k % 2) * 64 + 64, (k // 2) * H : (k // 2 + 1) * H
                ]
                eng = nc.sync if k % 2 == 0 else nc.scalar
                eng.dma_start(out=eflat[32 * k : 32 * k + 1, :], in_=src)

            # ---------------- adjacency ----------------
            t5 = singles.tile([128, 1], f32)
            nc.gpsimd.tensor_scalar_mul(t5, t_col, 5.0)
            # own adjacency columns [V, OWN] (needed by the contractions)
            sig_ord_o = work.tile([128, OWN], f32, tag="wk")
            nc.scalar.activation(sig_ord_o, to_row, AF.Sigmoid, bias=t5, scale=-5.0)
            sig_l_o = work.tile([128, OWN], f32, tag="wk")
            nc.scalar.activation(sig_l_o, adjlo_sb, AF.Sigmoid)
            adjp_o = work.tile([128, OWN], f32, tag="wk")
            nc.vector.tensor_tensor(out=adjp_o, in0=sig_l_o, in1=sig_ord_o, op=ALU.mult)
            adj_own = singles.tile([128, OWN], f32)
            nc.vector.tensor_tensor(out=adj_own, in0=adjp_o, in1=dmask_sb, op=ALU.mult)
            adj_own_bf = singles.tile([128, OWN], bf16)
            nc.gpsimd.tensor_copy(adj_own_bf, adj_own)
            # full adjacency (only for the adj output tensor)
            sig_ord = work.tile([128, V], f32, tag="wk2")
            nc.scalar.activation(sig_ord, t_row, AF.Sigmoid, bias=t5, scale=-5.0)
            sig_l = work.tile([128, V], f32, tag="wk2")
            nc.scalar.activation(sig_l, adjl_sb, AF.Sigmoid)
            adjp = work.tile([128, V], f32, tag="wk2")
            nc.gpsimd.tensor_tensor(out=adjp, in0=sig_l, in1=sig_ord, op=ALU.mult)
            adj_sb = singles.tile([128, V], f32)
            # zero the diagonal: keep where (row - col) != 0
            nc.gpsimd.affine_select(
                out=adj_sb,
                in_=adjp,
                pattern=[[-1, V]],
                compare_op=ALU.not_equal,
                fill=0.0,
                base=0,
                channel_multiplier=1,
            )
            nc.scalar.dma_start(out=d_adj_out.ap(), in_=adj_sb)

            # ---------------- main loop ----------------
            # weighted^T accumulators: [h, pair] columns, pair = b*16+il;
            # chunk m (pairs in [128m, 128m+128)) has its own PSUM bank so the
            # W2 stage for chunk 0 can start at the halfway point.
            waccs = [
                pswacc.tile([128, 128], f32, tag=f"wacc{m}", name=f"wacc{m}")
                for m in range(2)
            ]
            wsb = singles.tile([128, 256], f32)
            effsb = singles.tile([128, 256], f32)
            s_own = singles.tile([1, OWN], f32)
            s_rep = singles.tile([1, 128], f32)

            LAG = 3  # in 2-bank generation tiles
            NT = 2 * B  # 32 generation tiles, each covering 2 banks (8 pairs)
            hid_by_tile = {}

            def emit_contraction(kt):
                hid, b, q0 = hid_by_tile.pop(kt)
                for p in range(8):
                    il = 4 * q0 + p
                    gp = b * OWN + il
                    nc.tensor.matmul(
                        waccs[gp // 128][:, (gp % 128) : (gp % 128) + 1],
                        lhsT=hid[:, p * 128 : (p + 1) * 128],
                        rhs=adj_own_bf[:, il : il + 1],
                        start=True,
                        stop=True,
                    )

            def emit_w2(m):
                # weighted^T chunk -> effects rows [128m : 128m+128)
                nc.vector.tensor_copy(wsb[:, m * 128 : (m + 1) * 128], waccs[m])
                ps_eff = psgen.tile([128, 128], f32, tag="gen", name="ps_eff")
                nc.tensor.matmul(ps_eff, lhsT=s_rep, rhs=b2row, start=True, stop=False)
                nc.tensor.matmul(
                    ps_eff,
                    lhsT=wsb[:, m * 128 : (m + 1) * 128],
                    rhs=W2sb,
                    start=False,
                    stop=True,
                )
                if m == 0:
                    nc.scalar.copy(effsb[:, 0:128], ps_eff)
                else:
                    nc.vector.tensor_copy(effsb[:, 128:256], ps_eff)
                nc.sync.dma_start(
                    out=d_eff.ap()[m * 128 : (m + 1) * 128, :],
                    in_=effsb[:, m * 128 : (m + 1) * 128],
                )

            for kt in range(NT):
                b, half = divmod(kt, 2)  # half = 0 -> q 0,1 ; 1 -> q 2,3
                k32 = 32 * (b // 4)
                eoff = (b % 4) * (OWN * H)
                gen = psgen.tile([128, 1024], f32, tag="gen", name="gen")
                for s in range(2):
                    # cause part: C[b] = vars[b] @ W1a into 4 col blocks in one
                    # N=512 bf16 matmul (1 cycle/row vs 4 for fp32); both halves
                    # first so the stationary VT loads once per tile
                    nc.tensor.matmul(
                        gen[:, s * 512 : (s + 1) * 512],
                        lhsT=VT[:, b * V : (b + 1) * V],
                        rhs=W1a4,
                        start=True,
                        stop=False,
                    )
                for s in range(2):
                    q = 2 * half + s
                    # effect part (+b1): broadcast E rows across all j
                    # partitions via a K=32 matmul with an all-ones stationary
                    # block against zero-padded flattened E rows
                    nc.tensor.matmul(
                        gen[:, s * 512 : (s + 1) * 512],
                        lhsT=ones_sb[k32 : k32 + 32, :],
                        rhs=eflat[
                            k32 : k32 + 32, eoff + q * 512 : eoff + (q + 1) * 512
                        ],
                        start=False,
                        stop=True,
                        tile_position=(k32, 0),
                    )
                hid = hidp.tile([128, 1024], bf16, tag="hid", name="hid")
                if kt % 2 == 1:
                    nc.vector.tensor_scalar_max(hid, gen, 0.0)
                else:
                    nc.scalar.activation(hid, gen, AF.Relu)
                hid_by_tile[kt] = (hid, b, 2 * half)
                if kt == 1:
                    # column sums of adj (bias term), off the critical path
                    ps_s = psgen.tile([1, OWN], f32, tag="gen", name="ps_s")
                    nc.tensor.matmul(
                        ps_s, lhsT=ones_col, rhs=adj_own, start=True, stop=True
                    )
                    nc.vector.tensor_copy(s_own, ps_s)
                    for kk in range(8):
                        nc.gpsimd.tensor_copy(
                            s_rep[:, kk * OWN : (kk + 1) * OWN], s_own
                        )
                if kt >= LAG:
                    emit_contraction(kt - LAG)
                if kt == NT // 2 - 1 + LAG:
                    emit_w2(0)
            for kt in range(NT - LAG, NT):
                emit_contraction(kt)
            emit_w2(1)

    nc.finalize()
    return nc


def _get_program():
    if "nc" not in _CACHE:
        _CACHE["nc"] = _build_program()
    return _CACHE["nc"]


def _make_in_maps(inputs):
    variables = np.asarray(inputs["variables"], dtype=np.float32)
    adjl = np.asarray(inputs["adjacency_logits"], dtype=np.float32)
    tord = np.asarray(inputs["temporal_order"], dtype=np.float32)
    W1 = np.asarray(inputs["W1"], dtype=np.float32)
    b1 = np.asarray(inputs["b1"], dtype=np.float32)
    W2 = np.asarray(inputs["W2"], dtype=np.float32)
    b2 = np.asarray(inputs["b2"], dtype=np.float32)

    import ml_dtypes

    vt = np.ascontiguousarray(
        variables.reshape(B * V, H).T.astype(ml_dtypes.bfloat16)
    )  # [hh, (b,j)]

    in_maps = []
    for c in range(N_CORES):
        lo = OWN * c
        own = slice(lo, lo + OWN)
        pack = np.zeros((128, NPACK), dtype=np.float32)
        packs = np.zeros((128, NPACKS), dtype=np.float32)
        packs[:, OFF_ADJLO : OFF_ADJLO + OWN] = adjl[:, own]
        packs[:, OFF_TCOL] = tord
        packs[:, OFF_TOROW : OFF_TOROW + OWN] = np.broadcast_to(
            tord[own].reshape(1, OWN), (128, OWN)
        )
        dmask = np.ones((V, OWN), dtype=np.float32)
        dmask[lo + np.arange(OWN), np.arange(OWN)] = 0.0
        packs[:, OFF_DMASK : OFF_DMASK + OWN] = dmask
        packb = np.zeros((128, NPACKB), dtype=ml_dtypes.bfloat16)
        packb[:, OFF_VTO : OFF_VTO + B * OWN] = (
            variables[:, own, :].reshape(B * OWN, H).T.astype(ml_dtypes.bfloat16)
        )
        packb[:, OFF_W1B : OFF_W1B + H] = W1[H:].astype(ml_dtypes.bfloat16)
        packb[:, OFF_W1A4 : OFF_W1A4 + 4 * H] = np.tile(W1[:H], (1, 4)).astype(
            ml_dtypes.bfloat16
        )
        packb[:, OFF_B1R : OFF_B1R + H] = np.broadcast_to(
            b1.reshape(1, H), (128, H)
        ).astype(ml_dtypes.bfloat16)

        pack[:, OFF_W2 : OFF_W2 + H] = W2
        pack[:, OFF_ADJL : OFF_ADJL + V] = adjl
        pack[:, OFF_TROW : OFF_TROW + V] = np.broadcast_to(
            tord.reshape(1, V), (128, V)
        )
        pack[0, OFF_B2 : OFF_B2 + H] = b2
        in_maps.append({"pack1": pack, "packs": packs, "packb": packb, "vt": vt})
    return in_maps


def _run(inputs, trace=False, trace_kwargs=None):
    from concourse.bass_utils import run_bass_kernel_spmd

    nc = _get_program()
    in_maps = _make_in_maps(inputs)

    kw = {}
    if trace:
        kw["trace"] = True
        if trace_kwargs:
            kw["trace_kwargs"] = trace_kwargs
    res = run_bass_kernel_spmd(nc, in_maps, core_ids=list(range(N_CORES)), **kw)

    effects = np.zeros((B, V, H), dtype=np.float32)
    for c in range(N_CORES):
        eff_c = res.results[c]["eff"].reshape(B, OWN, H)
        effects[:, OWN * c : OWN * (c + 1), :] = eff_c
    adj = res.results[0]["adj_out"]
    return (effects, adj), res


def kernel(**inputs):
    (effects, adj), _ = _run(inputs, trace=False)
    return effects, adj
